# revision 1
# baseline (speedup 1.0000x reference)
"""Multi-head causal self-attention (B=2, T=2048, C=1024, H=16) on 8 trn2 cores.

Sharding: data-parallel over batch (2) x tensor-parallel over heads (4 groups
of 4 heads). Core c handles batch b=c//4, head group g=c%4:
  - column-parallel Wqkv slice (C, 768) -> Q/K/V for its 4 heads
  - flash-style causal attention computed in S^T orientation (k on
    partitions, q on free axis) so P^T feeds the PV matmul directly
  - row-parallel Wproj slice (256, C) -> partial projection output
  - ReduceScatter(add) over the 4 cores of the batch group; core with
    group index g ends with output rows [g*512, (g+1)*512)

All matmul operands are fp16 (values here are tiny: |S|<30, P in [0,1]),
accumulation is fp32 in PSUM. Softmax skips the max-subtraction (exp
argument bounded by ~5) and gets row sums from a ones-column appended to V.
"""

import os

import numpy as np

import concourse.bacc as bacc
import concourse.bass as bass
import concourse.mybir as mybir
import concourse.tile as tile
from concourse.bass_utils import run_bass_kernel_spmd

DEBUG = bool(int(os.environ.get("KERNEL_DEBUG", "0")))

F32 = mybir.dt.float32
F16 = mybir.dt.float16

B, T, C, H = 2, 2048, 1024, 16
HPC = 4                # heads per core
HD = 64                # head dim
CG = HPC * 3 * HD      # 768 qkv cols per core
PD = HPC * HD          # 256 proj rows per core
TT = T // 128          # 16 q/k tiles
KC = C // 128          # 8 contraction tiles
N_CORES = 8
NEG = -1.0e30


def _build():
    nc = bacc.Bacc(None, target_bir_lowering=False)

    x_in = nc.dram_tensor("x", [T, C], F32, kind="ExternalInput")
    wqkv_in = nc.dram_tensor("wqkv", [C, CG], F32, kind="ExternalInput")
    bqkv_in = nc.dram_tensor("bqkv", [1, CG], F32, kind="ExternalInput")
    wproj_in = nc.dram_tensor("wproj", [PD, C], F32, kind="ExternalInput")
    bproj_in = nc.dram_tensor("bproj", [1, C], F32, kind="ExternalInput")
    out_part = nc.dram_tensor("out_part", [T // 4, C], F32, kind="ExternalOutput")

    partial_d = nc.dram_tensor("partial_d", [T, C], F16)
    rsout_d = [nc.dram_tensor(f"rsout_d{i}", [T // 8, C], F16) for i in range(2)]

    dbg = {}
    if DEBUG:
        dbg["qkT"] = nc.dram_tensor("dbg_qkT", [128, 4 * T], F32, kind="ExternalOutput")
        dbg["v_aug"] = nc.dram_tensor(
            "dbg_v_aug", [128, TT * HPC * 65], F32, kind="ExternalOutput"
        )
        dbg["oT"] = nc.dram_tensor("dbg_oT", [64, HPC * T], F32, kind="ExternalOutput")
        dbg["xT"] = nc.dram_tensor("dbg_xT", [128, KC * T], F32, kind="ExternalOutput")
        dbg["partial"] = nc.dram_tensor("dbg_partial", [T, C], F32, kind="ExternalOutput")
        dbg["rowsum"] = nc.dram_tensor("dbg_rowsum", [HPC, T], F32, kind="ExternalOutput")
        dbg["recip"] = nc.dram_tensor("dbg_recip", [HPC, T], F32, kind="ExternalOutput")
        dbg["bc"] = nc.dram_tensor("dbg_bc", [64, T], F32, kind="ExternalOutput")
        dbg["ounorm"] = nc.dram_tensor("dbg_ounorm", [64, T], F32, kind="ExternalOutput")
        dbg["pt0"] = nc.dram_tensor("dbg_pt0", [128, 512], F32, kind="ExternalOutput")

    with tile.TileContext(nc) as tc:
        with (
            tc.tile_pool(name="cpool", bufs=1) as cpool,
            tc.tile_pool(name="main", bufs=1) as main,
            tc.tile_pool(name="stage", bufs=1) as stage,
        ):
            # ---------------- constants ----------------
            ident = cpool.tile([128, 128], F16)
            nc.gpsimd.memset(ident[:], 0.0)
            nc.gpsimd.affine_select(
                out=ident[:], in_=ident[:],
                compare_op=mybir.AluOpType.not_equal, fill=1.0,
                base=0, pattern=[[-1, 128]], channel_multiplier=1,
            )
            # S^T diag mask: keep (1) where q >= k, else 0 (x=k part, y=q free)
            mask_t = cpool.tile([128, 128], F16)
            nc.gpsimd.memset(mask_t[:], 1.0)
            nc.gpsimd.affine_select(
                out=mask_t[:], in_=mask_t[:],
                compare_op=mybir.AluOpType.is_ge, fill=0.0,
                base=0, pattern=[[1, 128]], channel_multiplier=-1,
            )
            ones_row = cpool.tile([1, 128], F16)
            nc.vector.memset(ones_row[:], 1.0)

            # qk bias vectors (128,1): [q01, q23, k01, k23] (host pre-permuted)
            qk_bias = cpool.tile([128, 4], F32)
            for i in range(4):
                nc.gpsimd.dma_start(
                    qk_bias[:, i : i + 1],
                    bqkv_in[0:1, i * 128 : (i + 1) * 128],
                )
            # v bias row (1, 256) f16 and proj bias row (1, 1024) f16
            vb_row = cpool.tile([1, HPC * HD], F16)
            nc.gpsimd.dma_start(vb_row[:], bqkv_in[0:1, 512:768])
            pb_row = cpool.tile([1, C], F16)
            nc.gpsimd.dma_start(pb_row[:], bproj_in[0:1, :])

            # ---------------- persistent tensors ----------------
            xT = main.tile([128, KC * T], F16)          # x^T: kc-th block at cols [kc*T, (kc+1)*T)
            qkT = main.tile([128, 4 * T], F16)          # [Q01; Q23; K01; K23] blocks of (128, T)
            v_aug = main.tile([128, TT * HPC * 65], F16)  # per tt: 4 heads x (64 V cols + ones)
            oT = main.tile([64, HPC * T], F16)          # per head: (64, T)
            wq16 = main.tile([128, KC * CG], F16)       # wqkv rows kc*128.. as f16
            wp16 = main.tile([64, HPC * C], F16)        # wproj rows per head at cols [h*C,(h+1)*C)
            vbias_rep = main.tile([128, HPC * HD], F16)
            pbias_rep = main.tile([128, C], F16)

            # weight loads (cast f32 -> f16 in DMA); host pre-permutes columns
            # (h t c) -> (t h c): [Q01|Q23|K01|K23|V0123] contiguous blocks
            for kc in range(KC):
                nc.gpsimd.dma_start(
                    wq16[:, kc * CG : (kc + 1) * CG],
                    wqkv_in[kc * 128 : (kc + 1) * 128, :],
                )
            for hh in range(HPC):
                nc.gpsimd.dma_start(
                    wp16[:, hh * C : (hh + 1) * C],
                    wproj_in[hh * 64 : (hh + 1) * 64, :],
                )

            # ones columns of v_aug (evacs only overwrite the 64-wide V blocks)
            nc.vector.memset(v_aug[:], 1.0)

            with tc.tile_pool(name="psAB", bufs=2, space="PSUM") as pAB:
                # bias replicas via K=1 broadcast matmuls
                bbp = pAB.tile([128, 256], F32, tag="bb", bufs=1)
                nc.tensor.matmul(bbp[:], ones_row[:, :], vb_row[:], start=True, stop=True)
                nc.vector.tensor_copy(vbias_rep[:], bbp[:])
                for ch in range(2):
                    bbp2 = pAB.tile([128, 512], F32, tag="bb", bufs=1)
                    nc.tensor.matmul(
                        bbp2[:], ones_row[:, :], pb_row[:, ch * 512 : (ch + 1) * 512],
                        start=True, stop=True,
                    )
                    nc.vector.tensor_copy(pbias_rep[:, ch * 512 : (ch + 1) * 512], bbp2[:])

                # ---------------- phase A: x load + transpose ----------------
                # x arrives as 4 big cast-DMAs into one staging tile; PE
                # transposes 128x128 blocks, 4 at a time into one PSUM bank,
                # evacuated by a single DVE copy each.
                x_r = x_in.rearrange("(t p) c -> p t c", p=128)
                for t4 in range(TT // 4):
                    x_q = stage.tile([128, 4 * C], F16, tag="xq", bufs=2)
                    nc.gpsimd.dma_start(
                        x_q[:], x_r[:, t4 * 4 : (t4 + 1) * 4, :]
                    )
                    for kc in range(KC):
                        xt_ps = pAB.tile([128, 512], F16, tag="xt")
                        for j in range(4):
                            nc.tensor.transpose(
                                xt_ps[:, j * 128 : (j + 1) * 128],
                                x_q[:, j * C + kc * 128 : j * C + (kc + 1) * 128],
                                ident[:],
                            )
                        nc.vector.tensor_copy(
                            xT[:, kc * T + t4 * 512 : kc * T + (t4 + 1) * 512], xt_ps[:]
                        )

                # ---------------- phase B: V then QKT ----------------
                # V: (T, 256) in tt tiles; scatter into 65-strided v_aug + bias
                for tt in range(TT):
                    ps = pAB.tile([128, 512], F32, tag="mm")
                    psv = ps[:, 0:256]
                    for kc in range(KC):
                        nc.tensor.matmul(
                            ps[:, 0:256],
                            xT[:, kc * T + tt * 128 : kc * T + (tt + 1) * 128],
                            wq16[:, kc * CG + 512 : kc * CG + 768],
                            start=(kc == 0),
                            stop=(kc == KC - 1),
                        )
                    vt = v_aug[:, tt * HPC * 65 : (tt + 1) * HPC * 65].rearrange(
                        "p (h c) -> p h c", c=65
                    )[:, :, 0:64]
                    nc.vector.scalar_tensor_tensor(
                        out=vt,
                        in0=psv.rearrange("p (h c) -> p h c", c=64),
                        scalar=1.0,
                        in1=vbias_rep[:].rearrange("p (h c) -> p h c", c=64),
                        op0=mybir.AluOpType.mult,
                        op1=mybir.AluOpType.add,
                    )

                # Q^T/K^T: out block i covers chans of 2 heads (128 rows);
                # head pair 0 (blocks 0,2) first so attention starts early
                for i in (0, 2, 1, 3):
                    for tch in range(T // 512):
                        ps = pAB.tile([128, 512], F32, tag="mm")
                        for kc in range(KC):
                            nc.tensor.matmul(
                                ps[:],
                                wq16[:, kc * CG + i * 128 : kc * CG + (i + 1) * 128],
                                xT[:, kc * T + tch * 512 : kc * T + (tch + 1) * 512],
                                start=(kc == 0),
                                stop=(kc == KC - 1),
                            )
                        nc.vector.tensor_scalar_add(
                            qkT[:, i * T + tch * 512 : i * T + (tch + 1) * 512],
                            ps[:],
                            qk_bias[:, i : i + 1],
                        )

            # ---------------- phase C: attention per head ----------------
            with tc.tile_pool(name="psC", bufs=1, space="PSUM") as pC:
                for l in range(HPC):
                    qT = qkT[64 * (l % 2) : 64 * (l % 2) + 64, (l // 2) * T : (l // 2 + 1) * T]
                    kT = qkT[64 * (l % 2) : 64 * (l % 2) + 64, (2 + l // 2) * T : (3 + l // 2) * T]
                    oT_ps = pC.tile([65, T], F32, tag="ot", bufs=1)
                    for kj in range(TT):
                        qlen = T - kj * 128
                        for ch in range((qlen + 1023) // 1024):
                            q0 = kj * 128 + ch * 1024
                            qn = min(1024, T - q0)
                            st = pC.tile([128, 1024], F32, tag="st", bufs=2)
                            for sc in range(0, qn, 512):
                                sn = min(512, qn - sc)
                                nc.tensor.matmul(
                                    st[:, sc : sc + sn],
                                    kT[:, kj * 128 : (kj + 1) * 128],
                                    qT[:, q0 + sc : q0 + sc + sn],
                                    start=True,
                                    stop=True,
                                )
                            pt = stage.tile([128, 1024], F16, tag="pt", bufs=4)
                            nc.scalar.activation(
                                pt[:, :qn], st[:, :qn],
                                mybir.ActivationFunctionType.Exp,
                                scale=0.125,
                            )
                            if ch == 0:
                                nc.gpsimd.tensor_mul(pt[:, :128], pt[:, :128], mask_t[:])
                            if DEBUG and l == 0 and kj == 0 and ch == 0:
                                nc.gpsimd.dma_start(dbg["pt0"][:], pt[:, :512])
                            vv = v_aug[:, kj * HPC * 65 + l * 65 : kj * HPC * 65 + (l + 1) * 65]
                            for qq in range(qn // 128):
                                qi = (q0 + qq * 128) // 128
                                # start=True clears has_written for the WHOLE
                                # bank: set it only on the first matmul that
                                # touches each 512-col bank (kj==0, qi%4==0).
                                nc.tensor.matmul(
                                    oT_ps[:, qi * 128 : (qi + 1) * 128],
                                    vv,
                                    pt[:, qq * 128 : (qq + 1) * 128],
                                    start=(kj == 0 and qi % 4 == 0),
                                    stop=(kj == qi),
                                )
                    # normalize: recip of rowsum row, broadcast to 64 partitions
                    rs_sb = stage.tile([1, T], F32, tag="rs_sb", bufs=2)
                    nc.vector.tensor_copy(rs_sb[:], oT_ps[64:65, :])
                    recip = stage.tile([1, T], F32, tag="recip", bufs=1)
                    nc.vector.reciprocal_approx_fast(recip[:], rs_sb[:])
                    recip16 = stage.tile([1, T], F16, tag="recip16", bufs=1)
                    nc.vector.tensor_copy(recip16[:], recip[:])
                    bc_sb = stage.tile([64, T], F16, tag="bcsb", bufs=2)
                    for ch in range(T // 512):
                        bc_ps = pC.tile([64, 512], F32, tag="st", bufs=2)
                        nc.tensor.matmul(
                            bc_ps[:],
                            ones_row[:, 0:64],
                            recip16[:, ch * 512 : (ch + 1) * 512],
                            start=True,
                            stop=True,
                        )
                        nc.vector.tensor_copy(bc_sb[:, ch * 512 : (ch + 1) * 512], bc_ps[:])
                    if DEBUG:
                        drs = stage.tile([1, T], F32, tag="drs", bufs=2)
                        nc.vector.tensor_copy(drs[:], rs_sb[:])
                        nc.gpsimd.dma_start(dbg["rowsum"][l : l + 1, :], drs[:])
                        nc.gpsimd.dma_start(dbg["recip"][l : l + 1, :], recip[:])
                        if l == 0:
                            nc.gpsimd.dma_start(dbg["bc"][:], bc_sb[:])
                            dou = stage.tile([64, T], F32, tag="dou", bufs=1)
                            nc.vector.tensor_copy(dou[:], oT_ps[0:64, :])
                            nc.gpsimd.dma_start(dbg["ounorm"][:], dou[:])
                    nc.vector.tensor_mul(
                        oT[:, l * T : (l + 1) * T], oT_ps[0:64, :], bc_sb[:]
                    )

            # ---------------- phase D: projection + chunked reduce-scatter ----
            part_r = partial_d.rearrange("(a p) c -> p a c", p=128)
            with tc.tile_pool(name="psD", bufs=2, space="PSUM") as pD:
                for cq in range(4):
                    part4 = stage.tile([128, 4 * C], F16, tag="part", bufs=1)
                    for j in range(4):
                        tt = cq * 4 + j
                        pp = pD.tile([128, C], F32, tag="pp")
                        for nch in range(2):
                            for hh in range(HPC):
                                nc.tensor.matmul(
                                    pp[:, nch * 512 : (nch + 1) * 512],
                                    oT[:, hh * T + tt * 128 : hh * T + (tt + 1) * 128],
                                    wp16[:, hh * C + nch * 512 : hh * C + (nch + 1) * 512],
                                    start=(hh == 0),
                                    stop=(hh == HPC - 1),
                                )
                        nc.vector.scalar_tensor_tensor(
                            out=part4[:, j * C : (j + 1) * C],
                            in0=pp[:],
                            scalar=1.0,
                            in1=pbias_rep[:],
                            op0=mybir.AluOpType.mult,
                            op1=mybir.AluOpType.add,
                        )
                    nc.sync.dma_start(
                        part_r[:, cq * 4 : (cq + 1) * 4, :],
                        part4[:].rearrange("p (a c) -> p a c", a=4),
                    )
                    if cq % 2 == 1:
                        hf = cq // 2
                        nc.gpsimd.collective_compute(
                            "ReduceScatter",
                            mybir.AluOpType.add,
                            replica_groups=[[0, 1, 2, 3], [4, 5, 6, 7]],
                            ins=[partial_d[hf * 1024 : (hf + 1) * 1024, :]],
                            outs=[rsout_d[hf][:]],
                        )
                        for j2 in range(2):
                            rsb = stage.tile([128, C], F32, tag="rsb", bufs=2)
                            nc.gpsimd.dma_start(
                                rsb[:], rsout_d[hf][j2 * 128 : (j2 + 1) * 128, :]
                            )
                            nc.sync.dma_start(
                                out_part[hf * 256 + j2 * 128 : hf * 256 + (j2 + 1) * 128, :],
                                rsb[:],
                            )

            if DEBUG:
                nc.gpsimd.dma_start(dbg["qkT"][:], qkT[:])
                nc.gpsimd.dma_start(dbg["v_aug"][:], v_aug[:])
                nc.gpsimd.dma_start(dbg["oT"][:], oT[:])
                nc.gpsimd.dma_start(dbg["xT"][:], xT[:])
                nc.gpsimd.dma_start(dbg["partial"][:], partial_d[:])


    nc.finalize()
    return nc


_NC = None


def _get_nc():
    global _NC
    if _NC is None:
        _NC = _build()
    return _NC


def _make_in_maps(x, Wqkv, bqkv, Wproj, bproj):
    x = np.asarray(x, dtype=np.float32)
    Wqkv = np.asarray(Wqkv, dtype=np.float32)
    bqkv = np.asarray(bqkv, dtype=np.float32)
    Wproj = np.asarray(Wproj, dtype=np.float32)
    bproj = np.asarray(bproj, dtype=np.float32)
    zeros_c = np.zeros((1, C), np.float32)

    def perm_qkv(w):
        # (..., h*192 + t*64 + c) -> (..., t*256 + h*64 + c)
        s = w.shape[:-1]
        return np.ascontiguousarray(
            w.reshape(*s, HPC, 3, HD).swapaxes(-3, -2).reshape(*s, CG)
        )

    in_maps = []
    for c in range(N_CORES):
        b, g = divmod(c, 4)
        in_maps.append(
            {
                "x": np.ascontiguousarray(x[b]),
                "wqkv": perm_qkv(Wqkv[:, g * CG : (g + 1) * CG]),
                "bqkv": perm_qkv(bqkv[g * CG : (g + 1) * CG]).reshape(1, CG),
                "wproj": np.ascontiguousarray(Wproj[g * PD : (g + 1) * PD, :]),
                "bproj": bproj.reshape(1, C) if g == 0 else zeros_c,
            }
        )
    return in_maps


def _run(in_maps, trace=False):
    nc = _get_nc()
    return run_bass_kernel_spmd(nc, in_maps, list(range(N_CORES)), trace=trace)


def kernel(x, Wqkv, bqkv, Wproj, bproj):
    in_maps = _make_in_maps(x, Wqkv, bqkv, Wproj, bproj)
    res = _run(in_maps)
    out = np.empty((B, T, C), np.float32)
    for c in range(N_CORES):
        b, g = divmod(c, 4)
        op = res.results[c]["out_part"]
        for hf in range(2):
            out[b, hf * 1024 + g * 256 : hf * 1024 + (g + 1) * 256, :] = op[
                hf * 256 : (hf + 1) * 256
            ]
    return out



# revision 7
# speedup vs baseline: 1.5235x; 1.5235x over previous
"""Multi-head causal self-attention (B=2, T=2048, C=1024, H=16) on 8 trn2 cores.

Sharding: data-parallel over batch (2) x tensor-parallel over heads (4 groups
of 4 heads). Core c handles batch b=c//4, head group g=c%4.

Key structure (per core):
  - x is pre-transposed and pre-cast to f16 on the host (window-major
    layout) so there is no on-device transpose phase and every DMA is
    cast-free. fp8 was evaluated for the QKV projection (DoubleRow) but its
    quantization noise exceeds the 2e-2 budget, so everything stays f16.
  - Attention in S^T orientation (k on partitions, q free), f16 operands.
    The causal mask is a PE add-matmul into the S accumulation group
    (-200 strict-upper-tri stationary x identity moving).
  - Rowsums come from a ones-column appended to V; recip on DVE;
    partition_broadcast on Pool.
  - Output projection (row-parallel Wproj, bias via a 65th ones-row in oT)
    is pipelined per 512-row sub-chunk with attention of the next sub-chunk,
    feeding chunked ReduceScatter collectives that overlap compute.
  - All DMAs are cast-free (host pre-casts) and issue via HWDGE (nc.sync).
"""

import os

import numpy as np

import concourse.bacc as bacc
import concourse.bass as bass
import concourse.mybir as mybir
import concourse.tile as tile
from concourse.bass_utils import run_bass_kernel_spmd

DEBUG = bool(int(os.environ.get("KERNEL_DEBUG", "0")))

F32 = mybir.dt.float32
F16 = mybir.dt.float16

B, T, C, H = 2, 2048, 1024, 16
HPC = 4                 # heads per core
HD = 64                 # head dim
CG = HPC * 3 * HD       # 768 qkv cols per core
KC = 8                  # f16 contraction chunks (128 channels each)
TT = T // 128           # 16 k tiles
NSC = T // 512          # 4 q sub-chunks
N_CORES = 8
EXP_SCALE = 0.125

# reduce-scatter groups as (row_start, row_end); each core keeps len/4 rows
RS_GROUPS = [(0, 1024), (1024, 1536), (1536, 2048)]


def _build():
    nc = bacc.Bacc(None, target_bir_lowering=False)

    x16_in = nc.dram_tensor("x16", [128, 4 * KC * 512], F16, kind="ExternalInput")
    w16_in = nc.dram_tensor("w16", [128, KC * CG], F16, kind="ExternalInput")
    qkb_in = nc.dram_tensor("qkb", [1, 512], F16, kind="ExternalInput")
    vb_in = nc.dram_tensor("vb", [1, 256], F16, kind="ExternalInput")
    wpa_in = nc.dram_tensor("wpa", [65, HPC * C], F16, kind="ExternalInput")
    out_part = nc.dram_tensor("out_part", [T // 4, C], F16, kind="ExternalOutput")

    partial_d = nc.dram_tensor("partial_d", [T, C], F16)
    rsout_d = [
        nc.dram_tensor(f"rsout_d{i}", [(r1 - r0) // 4, C], F16)
        for i, (r0, r1) in enumerate(RS_GROUPS)
    ]

    dbg = {}
    if DEBUG:
        dbg["qkT"] = nc.dram_tensor("dbg_qkT", [128, 4 * T], F16, kind="ExternalOutput")
        dbg["v_aug"] = nc.dram_tensor(
            "dbg_v_aug", [128, TT * HPC * 65], F16, kind="ExternalOutput"
        )
        dbg["oT"] = nc.dram_tensor("dbg_oT", [65, HPC * 512 * 2], F16, kind="ExternalOutput")
        dbg["partial"] = nc.dram_tensor("dbg_partial", [T, C], F16, kind="ExternalOutput")

    with tile.TileContext(nc) as tc:
        with (
            tc.tile_pool(name="cpool", bufs=1) as cpool,
            tc.tile_pool(name="main", bufs=1) as main,
            tc.tile_pool(name="stage", bufs=1) as stage,
            tc.tile_pool(name="ps", bufs=1, space="PSUM") as ps,
        ):
            # ---------------- constants ----------------
            ones_row = cpool.tile([1, 512], F16)
            nc.vector.memset(ones_row[:], 1.0)
            # mask stationary: mstat[f, p] = -200 where p > f else 0
            mstat = cpool.tile([128, 128], F16)
            nc.gpsimd.memset(mstat[:], -200.0)
            nc.gpsimd.affine_select(
                out=mstat[:], in_=mstat[:],
                compare_op=mybir.AluOpType.is_ge, fill=0.0,
                base=-1, pattern=[[1, 128]], channel_multiplier=-1,
            )
            # mask moving: identity
            mmov = cpool.tile([128, 128], F16)
            nc.gpsimd.memset(mmov[:], 0.0)
            nc.gpsimd.affine_select(
                out=mmov[:], in_=mmov[:],
                compare_op=mybir.AluOpType.not_equal, fill=1.0,
                base=0, pattern=[[-1, 128]], channel_multiplier=1,
            )

            # ---------------- persistent tensors ----------------
            x16 = main.tile([128, 4 * KC * 512], F16)   # [w][kc][512]
            w16 = main.tile([128, KC * CG], F16)        # [kc][768]
            qkb = main.tile([1, 512], F16)
            vb = main.tile([1, 256], F16)
            wpa = main.tile([65, HPC * C], F16)
            qkT = main.tile([128, 4 * T], F16)             # [Q01;Q23;K01;K23] x T
            v_aug = main.tile([128, TT * HPC * 65], F16)   # per (tt,h): 64 V + ones col
            oT_sb = [
                main.tile([65, HPC * 512], F16, name=f"oT_sb{i}") for i in range(2)
            ]

            nc.vector.memset(v_aug[:], 1.0)  # ones columns give softmax rowsums
            for buf in oT_sb:
                nc.vector.memset(buf[64:65, :], 1.0)

            # ---------------- input DMAs (all cast-free, HWDGE) ----------
            w16_r = w16[:].rearrange("p (kc m) -> p kc m", kc=KC)
            w16_in_r = w16_in[:].rearrange("p (kc m) -> p kc m", kc=KC)
            nc.sync.dma_start(w16_r[:, :, 512:768], w16_in_r[:, :, 512:768])
            nc.sync.dma_start(
                x16[:, 0:4096], x16_in[:, 0:4096]
            )
            nc.sync.dma_start(qkb[:], qkb_in[:])
            nc.sync.dma_start(vb[:], vb_in[:])
            nc.sync.dma_start(w16_r[:, :, 0:512], w16_in_r[:, :, 0:512])
            for w in range(1, 4):
                nc.sync.dma_start(
                    x16[:, w * 4096 : (w + 1) * 4096],
                    x16_in[:, w * 4096 : (w + 1) * 4096],
                )
            nc.sync.dma_start(wpa[:], wpa_in[:])

            # ---------------- emit helpers ----------------
            def x16_w(w):
                # [128, kc, 512] view of window w
                return x16[:, w * 4096 : (w + 1) * 4096].rearrange(
                    "p (kc t) -> p kc t", kc=KC
                )


            def emit_v(tt):
                w, tloc = divmod(tt, 4)
                pp = ps.tile([128, 512], F32, tag="mm", bufs=3)
                for kc in range(KC):
                    nc.tensor.matmul(
                        pp[:, 0:256],
                        x16_w(w)[:, kc, tloc * 128 : (tloc + 1) * 128],
                        w16_r[:, kc, 512:768],
                        start=(kc == 0), stop=False,
                    )
                nc.tensor.matmul(
                    pp[:, 0:256], ones_row[:, 0:128], vb[:],
                    start=False, stop=True, skip_group_check=True,
                )
                vt = v_aug[:, tt * HPC * 65 : (tt + 1) * HPC * 65].rearrange(
                    "p (h c) -> p h c", c=65
                )[:, :, 0:64]
                nc.scalar.activation(
                    vt,
                    pp[:, 0:256].rearrange("p (h c) -> p h c", c=64),
                    mybir.ActivationFunctionType.Copy,
                )

            def emit_qk(i, tch):
                pp = ps.tile([128, 512], F32, tag="mm", bufs=3)
                for kc in range(KC):
                    nc.tensor.matmul(
                        pp[:],
                        w16_r[:, kc, i * 128 : (i + 1) * 128],
                        x16_w(tch)[:, kc, :],
                        start=(kc == 0), stop=False,
                    )
                nc.tensor.matmul(
                    pp[:], qkb[:, i * 128 : (i + 1) * 128], ones_row[:],
                    start=False, stop=True, skip_group_check=True,
                )
                dst = qkT[:, i * T + tch * 512 : i * T + (tch + 1) * 512]
                if i < 2:
                    nc.vector.tensor_copy(dst, pp[:])
                else:
                    nc.scalar.activation(
                        dst, pp[:], mybir.ActivationFunctionType.Copy
                    )

            # per (sc, h) attention state
            ot_tiles = {}
            rs_tiles = {}

            def emit_att_head(sc, h):
                qT = qkT[64 * (h % 2) : 64 * (h % 2) + 64, (h // 2) * T : (h // 2 + 1) * T]
                kT = qkT[64 * (h % 2) : 64 * (h % 2) + 64, (2 + h // 2) * T : (3 + h // 2) * T]
                oT_ps = ps.tile([65, 512], F32, tag="ot", bufs=5)
                ot_tiles[(sc, h)] = oT_ps
                n_kj = (sc + 1) * 4
                for kj in range(n_kj):
                    q_off = max(0, kj * 128 - sc * 512)
                    cols = 512 - q_off
                    diag = kj >= sc * 4
                    st = ps.tile([128, 512], F32, tag="mm", bufs=3)
                    nc.tensor.matmul(
                        st[:, :cols],
                        kT[:, kj * 128 : (kj + 1) * 128],
                        qT[:, sc * 512 + q_off : (sc + 1) * 512],
                        start=True, stop=not diag,
                    )
                    if diag:
                        nc.tensor.matmul(
                            st[:, 0:128], mstat[:], mmov[:],
                            start=False, stop=True, skip_group_check=True,
                        )
                    pt = stage.tile([128, 512], F16, tag="pt", bufs=4)
                    nc.scalar.activation(
                        pt[:, :cols], st[:, :cols],
                        mybir.ActivationFunctionType.Exp,
                        scale=EXP_SCALE,
                    )
                    vv = v_aug[:, (kj * HPC + h) * 65 : (kj * HPC + h + 1) * 65]
                    nc.tensor.matmul(
                        oT_ps[:, q_off:512],
                        vv,
                        pt[:, :cols],
                        start=(kj == 0),
                        stop=(kj == n_kj - 1),
                    )

            def emit_chain(sc):
                # recip + broadcast + normalize for all 4 heads of sc
                for h in range(HPC):
                    oT_ps = ot_tiles[(sc, h)]
                    rs = stage.tile([1, 512], F32, tag="rs", bufs=4)
                    nc.vector.tensor_copy(rs[:], oT_ps[64:65, :])
                    recip = stage.tile([1, 512], F32, tag="recip", bufs=4)
                    nc.vector.reciprocal_approx_fast(recip[:], rs[:])
                    bc = stage.tile([64, 512], F32, tag="bc", bufs=4)
                    nc.gpsimd.partition_broadcast(bc[:], recip[:])
                    dst = oT_sb[sc % 2][0:64, h * 512 : (h + 1) * 512]
                    nc.vector.tensor_mul(dst, oT_ps[0:64, :], bc[:])

            def emit_proj_piece(sc, j):
                # project rows [sc*512 + j*128, +128)
                r0 = sc * 512 + j * 128
                oT_cur = oT_sb[sc % 2]
                for nch in range(2):
                    pp = ps.tile([128, 512], F32, tag="mm", bufs=3)
                    for hh in range(HPC):
                        nc.tensor.matmul(
                            pp[:],
                            oT_cur[:, hh * 512 + j * 128 : hh * 512 + (j + 1) * 128],
                            wpa[:, hh * C + nch * 512 : hh * C + (nch + 1) * 512],
                            start=(hh == 0), stop=(hh == HPC - 1),
                        )
                    pst = stage.tile([128, 512], F16, tag="pst", bufs=4)
                    nc.vector.tensor_copy(pst[:], pp[:])
                    nc.sync.dma_start(
                        partial_d[r0 : r0 + 128, nch * 512 : (nch + 1) * 512],
                        pst[:],
                    )

            def emit_rs(gi):
                r0, r1 = RS_GROUPS[gi]
                nc.gpsimd.collective_compute(
                    "ReduceScatter",
                    mybir.AluOpType.add,
                    replica_groups=[[0, 1, 2, 3], [4, 5, 6, 7]],
                    ins=[partial_d[r0:r1, :]],
                    outs=[rsout_d[gi][:]],
                )
                og = sum((b1 - b0) // 4 for (b0, b1) in RS_GROUPS[:gi])
                ln4 = (r1 - r0) // 4
                nc.sync.dma_start(out_part[og : og + ln4, :], rsout_d[gi][:])

            # ---------------- schedule ----------------
            for tt in range(8):
                emit_v(tt)
            for tch in (0, 1):
                for i in range(4):
                    emit_qk(i, tch)

            # att sc0 (no pending proj work)
            for h in range(HPC):
                emit_att_head(0, h)
            emit_chain(0)

            # att sc1 with proj(sc0) interleaved
            for h in range(HPC):
                emit_att_head(1, h)
                emit_proj_piece(0, h)
            emit_chain(1)

            # rest of phase B
            for tt in range(8, 16):
                emit_v(tt)
            for tch in (2, 3):
                for i in range(4):
                    emit_qk(i, tch)

            # att sc2 with proj(sc1); RS group 0 (rows 0..1023) after last piece
            for h in range(HPC):
                emit_att_head(2, h)
                emit_proj_piece(1, h)
            emit_rs(0)
            emit_chain(2)

            # att sc3 with proj(sc2); RS group 1 (rows 1024..1535)
            for h in range(HPC):
                emit_att_head(3, h)
                emit_proj_piece(2, h)
            emit_rs(1)
            emit_chain(3)

            # tail: proj(sc3) + RS group 2
            for j in range(4):
                emit_proj_piece(3, j)
            emit_rs(2)

            if DEBUG:
                nc.sync.dma_start(dbg["qkT"][:], qkT[:])
                nc.sync.dma_start(dbg["v_aug"][:], v_aug[:])
                nc.sync.dma_start(dbg["oT"][:, 0 : HPC * 512], oT_sb[0][:])
                nc.sync.dma_start(dbg["oT"][:, HPC * 512 :], oT_sb[1][:])
                nc.sync.dma_start(dbg["partial"][:], partial_d[:])

    nc.finalize()
    return nc


_NC = None


def _get_nc():
    global _NC
    if _NC is None:
        _NC = _build()
    return _NC


def _perm_qkv(w):
    # (..., h*192 + t*64 + c) -> (..., t*256 + h*64 + c)
    s = w.shape[:-1]
    return np.ascontiguousarray(
        w.reshape(*s, HPC, 3, HD).swapaxes(-3, -2).reshape(*s, CG)
    )


def _make_in_maps(x, Wqkv, bqkv, Wproj, bproj):
    x = np.asarray(x, dtype=np.float32)
    Wqkv = np.asarray(Wqkv, dtype=np.float32)
    bqkv = np.asarray(bqkv, dtype=np.float32)
    Wproj = np.asarray(Wproj, dtype=np.float32)
    bproj = np.asarray(bproj, dtype=np.float32)

    in_maps = []
    for c in range(N_CORES):
        b, g = divmod(c, 4)
        # x16: [p, w, kc, t] window-major transposed layout
        xT = x[b].T  # (C, T)
        x16 = np.ascontiguousarray(
            xT.reshape(KC, 128, 4, 512).transpose(1, 2, 0, 3).reshape(128, -1)
        ).astype(np.float16)
        wp_ = _perm_qkv(Wqkv[:, g * CG : (g + 1) * CG])
        w16 = np.ascontiguousarray(
            wp_.reshape(KC, 128, CG).transpose(1, 0, 2).reshape(128, -1)
        ).astype(np.float16)
        bq = _perm_qkv(bqkv[g * CG : (g + 1) * CG])
        qkb = np.ascontiguousarray(bq[:512].reshape(1, 512)).astype(np.float16)
        vb = bq[512:768].reshape(1, 256).astype(np.float16)
        wpa = np.zeros((65, HPC * C), np.float32)
        for hh in range(HPC):
            wpa[0:64, hh * C : (hh + 1) * C] = Wproj[
                g * 256 + hh * 64 : g * 256 + (hh + 1) * 64, :
            ]
        if g == 0:
            wpa[64, 0:C] = bproj
        in_maps.append(
            {
                "x16": x16,
                "w16": w16,
                "qkb": qkb,
                "vb": vb,
                "wpa": wpa.astype(np.float16),
            }
        )
    return in_maps


def _run(in_maps, trace=False):
    nc = _get_nc()
    return run_bass_kernel_spmd(nc, in_maps, list(range(N_CORES)), trace=trace)


def kernel(x, Wqkv, bqkv, Wproj, bproj):
    in_maps = _make_in_maps(x, Wqkv, bqkv, Wproj, bproj)
    res = _run(in_maps)
    out = np.empty((B, T, C), np.float32)
    for c in range(N_CORES):
        b, g = divmod(c, 4)
        op = res.results[c]["out_part"].astype(np.float32)
        og = 0
        for r0, r1 in RS_GROUPS:
            ln4 = (r1 - r0) // 4
            out[b, r0 + g * ln4 : r0 + (g + 1) * ln4, :] = op[og : og + ln4]
            og += ln4
    return out


# revision 8
# speedup vs baseline: 1.6583x; 1.0885x over previous
"""Multi-head causal self-attention (B=2, T=2048, C=1024, H=16) on 8 trn2 cores.

Sharding: data-parallel over batch (2) x tensor-parallel over heads (4 groups
of 4 heads). Core c handles batch b=c//4, head group g=c%4.

Key structure (per core):
  - x is pre-transposed and pre-cast to f16 on the host (window-major
    layout) so there is no on-device transpose phase and every DMA is
    cast-free. fp8 was evaluated for the QKV projection (DoubleRow) but its
    quantization noise exceeds the 2e-2 budget, so everything stays f16.
  - Attention in S^T orientation (k on partitions, q free), f16 operands.
    The causal mask is a PE add-matmul into the S accumulation group
    (-200 strict-upper-tri stationary x identity moving).
  - Rowsums come from a ones-column appended to V; recip on DVE;
    partition_broadcast on Pool.
  - Output projection (row-parallel Wproj, bias via a 65th ones-row in oT)
    is pipelined per 512-row sub-chunk with attention of the next sub-chunk,
    feeding chunked ReduceScatter collectives that overlap compute.
  - All DMAs are cast-free (host pre-casts) and issue via HWDGE (nc.sync).
"""

import os

import numpy as np

import concourse.bacc as bacc
import concourse.bass as bass
import concourse.mybir as mybir
import concourse.tile as tile
from concourse.bass_utils import run_bass_kernel_spmd

DEBUG = bool(int(os.environ.get("KERNEL_DEBUG", "0")))

F32 = mybir.dt.float32
F16 = mybir.dt.float16

B, T, C, H = 2, 2048, 1024, 16
HPC = 4                 # heads per core
HD = 64                 # head dim
CG = HPC * 3 * HD       # 768 qkv cols per core
KC = 8                  # f16 contraction chunks (128 channels each)
TT = T // 128           # 16 k tiles
NSC = T // 512          # 4 q sub-chunks
N_CORES = 8
EXP_SCALE = 0.125

# reduce-scatter groups as (row_start, row_end); each core keeps len/4 rows
RS_GROUPS = [(0, 1024), (1024, 1536), (1536, 2048)]


def _build():
    nc = bacc.Bacc(None, target_bir_lowering=False)

    x16_in = nc.dram_tensor("x16", [128, 4 * KC * 512], F16, kind="ExternalInput")
    w16_in = nc.dram_tensor("w16", [128, KC * CG], F16, kind="ExternalInput")
    qkb_in = nc.dram_tensor("qkb", [128, 4], F32, kind="ExternalInput")
    vb_in = nc.dram_tensor("vb", [1, 256], F16, kind="ExternalInput")
    wpa_in = nc.dram_tensor("wpa", [65, HPC * C], F16, kind="ExternalInput")
    out_part = nc.dram_tensor("out_part", [T // 4, C], F16, kind="ExternalOutput")

    partial_d = nc.dram_tensor("partial_d", [T, C], F16)
    rsout_d = [
        nc.dram_tensor(f"rsout_d{i}", [(r1 - r0) // 4, C], F16)
        for i, (r0, r1) in enumerate(RS_GROUPS)
    ]

    dbg = {}
    if DEBUG:
        dbg["qkT"] = nc.dram_tensor("dbg_qkT", [128, 4 * T], F16, kind="ExternalOutput")
        dbg["v_aug"] = nc.dram_tensor(
            "dbg_v_aug", [128, TT * HPC * 65], F16, kind="ExternalOutput"
        )
        dbg["oT"] = nc.dram_tensor("dbg_oT", [65, HPC * 512 * 2], F16, kind="ExternalOutput")
        dbg["partial"] = nc.dram_tensor("dbg_partial", [T, C], F16, kind="ExternalOutput")

    with tile.TileContext(nc) as tc:
        with (
            tc.tile_pool(name="cpool", bufs=1) as cpool,
            tc.tile_pool(name="main", bufs=1) as main,
            tc.tile_pool(name="stage", bufs=1) as stage,
            tc.tile_pool(name="ps", bufs=1, space="PSUM") as ps,
        ):
            # ---------------- constants ----------------
            ones_row = cpool.tile([1, 512], F16)
            nc.vector.memset(ones_row[:], 1.0)
            # mask stationary: mstat[f, p] = -200 where p > f else 0
            mstat = cpool.tile([128, 128], F16)
            nc.gpsimd.memset(mstat[:], -200.0)
            nc.gpsimd.affine_select(
                out=mstat[:], in_=mstat[:],
                compare_op=mybir.AluOpType.is_ge, fill=0.0,
                base=-1, pattern=[[1, 128]], channel_multiplier=-1,
            )
            # mask moving: identity
            mmov = cpool.tile([128, 128], F16)
            nc.gpsimd.memset(mmov[:], 0.0)
            nc.gpsimd.affine_select(
                out=mmov[:], in_=mmov[:],
                compare_op=mybir.AluOpType.not_equal, fill=1.0,
                base=0, pattern=[[-1, 128]], channel_multiplier=1,
            )

            # ---------------- persistent tensors ----------------
            x16 = main.tile([128, 4 * KC * 512], F16)   # [w][kc][512]
            w16 = main.tile([128, KC * CG], F16)        # [kc][768]
            qkb = main.tile([128, 4], F32)
            vb = main.tile([1, 256], F16)
            wpa = main.tile([65, HPC * C], F16)
            qkT = main.tile([128, 4 * T], F16)             # [Q01;Q23;K01;K23] x T
            v_aug = main.tile([128, TT * HPC * 65], F16)   # per (tt,h): 64 V + ones col
            oT_sb = [
                main.tile([65, HPC * 512], F16, name=f"oT_sb{i}") for i in range(2)
            ]

            nc.vector.memset(v_aug[:], 1.0)  # ones columns give softmax rowsums
            for buf in oT_sb:
                nc.vector.memset(buf[64:65, :], 1.0)

            # ---------------- input DMAs (all cast-free, HWDGE) ----------
            w16_r = w16[:].rearrange("p (kc m) -> p kc m", kc=KC)
            w16_in_r = w16_in[:].rearrange("p (kc m) -> p kc m", kc=KC)
            nc.sync.dma_start(w16_r[:, :, 512:768], w16_in_r[:, :, 512:768])
            nc.sync.dma_start(
                x16[:, 0:4096], x16_in[:, 0:4096]
            )
            nc.sync.dma_start(qkb[:], qkb_in[:])
            nc.sync.dma_start(vb[:], vb_in[:])
            nc.sync.dma_start(w16_r[:, :, 0:512], w16_in_r[:, :, 0:512])
            for w in range(1, 4):
                nc.sync.dma_start(
                    x16[:, w * 4096 : (w + 1) * 4096],
                    x16_in[:, w * 4096 : (w + 1) * 4096],
                )
            nc.sync.dma_start(wpa[:], wpa_in[:])

            # ---------------- emit helpers ----------------
            def x16_w(w):
                # [128, kc, 512] view of window w
                return x16[:, w * 4096 : (w + 1) * 4096].rearrange(
                    "p (kc t) -> p kc t", kc=KC
                )


            def emit_v(tt):
                w, tloc = divmod(tt, 4)
                pp = ps.tile([128, 512], F32, tag="mm", bufs=4)
                for kc in range(KC):
                    nc.tensor.matmul(
                        pp[:, 0:256],
                        x16_w(w)[:, kc, tloc * 128 : (tloc + 1) * 128],
                        w16_r[:, kc, 512:768],
                        start=(kc == 0), stop=False,
                    )
                nc.tensor.matmul(
                    pp[:, 0:256], ones_row[:, 0:128], vb[:],
                    start=False, stop=True, skip_group_check=True,
                )
                vt = v_aug[:, tt * HPC * 65 : (tt + 1) * HPC * 65].rearrange(
                    "p (h c) -> p h c", c=65
                )[:, :, 0:64]
                nc.scalar.activation(
                    vt,
                    pp[:, 0:256].rearrange("p (h c) -> p h c", c=64),
                    mybir.ActivationFunctionType.Copy,
                )

            def emit_qk(i, tch):
                pp = ps.tile([128, 512], F32, tag="mm", bufs=4)
                for kc in range(KC):
                    nc.tensor.matmul(
                        pp[:],
                        w16_r[:, kc, i * 128 : (i + 1) * 128],
                        x16_w(tch)[:, kc, :],
                        start=(kc == 0), stop=(kc == KC - 1),
                    )
                dst = qkT[:, i * T + tch * 512 : i * T + (tch + 1) * 512]
                nc.vector.tensor_scalar_add(dst, pp[:], qkb[:, i : i + 1])

            # per (sc, h) attention state
            ot_tiles = {}
            rs_tiles = {}

            def emit_att_head(sc, h):
                qT = qkT[64 * (h % 2) : 64 * (h % 2) + 64, (h // 2) * T : (h // 2 + 1) * T]
                kT = qkT[64 * (h % 2) : 64 * (h % 2) + 64, (2 + h // 2) * T : (3 + h // 2) * T]
                oT_ps = ps.tile([65, 512], F32, tag="ot", bufs=4)
                ot_tiles[(sc, h)] = oT_ps
                n_kj = (sc + 1) * 4
                for kj in range(n_kj):
                    q_off = max(0, kj * 128 - sc * 512)
                    cols = 512 - q_off
                    diag = kj >= sc * 4
                    st = ps.tile([128, 512], F32, tag="mm", bufs=4)
                    nc.tensor.matmul(
                        st[:, :cols],
                        kT[:, kj * 128 : (kj + 1) * 128],
                        qT[:, sc * 512 + q_off : (sc + 1) * 512],
                        start=True, stop=not diag,
                    )
                    if diag:
                        nc.tensor.matmul(
                            st[:, 0:128], mstat[:], mmov[:],
                            start=False, stop=True, skip_group_check=True,
                        )
                    pt = stage.tile([128, 512], F16, tag="pt", bufs=4)
                    nc.scalar.activation(
                        pt[:, :cols], st[:, :cols],
                        mybir.ActivationFunctionType.Exp,
                        scale=EXP_SCALE,
                    )
                    vv = v_aug[:, (kj * HPC + h) * 65 : (kj * HPC + h + 1) * 65]
                    nc.tensor.matmul(
                        oT_ps[:, q_off:512],
                        vv,
                        pt[:, :cols],
                        start=(kj == 0),
                        stop=(kj == n_kj - 1),
                    )
                # normalize chain for this head (DVE + Pool), frees oT_ps
                rs = stage.tile([1, 512], F32, tag="rs", bufs=4)
                nc.vector.tensor_copy(rs[:], oT_ps[64:65, :])
                recip = stage.tile([1, 512], F32, tag="recip", bufs=4)
                nc.vector.reciprocal_approx_fast(recip[:], rs[:])
                bc = stage.tile([64, 512], F32, tag="bc", bufs=4)
                nc.gpsimd.partition_broadcast(bc[:], recip[:])
                nc.vector.tensor_mul(
                    oT_sb[sc % 2][0:64, h * 512 : (h + 1) * 512],
                    oT_ps[0:64, :], bc[:],
                )

            def emit_proj_piece(sc, j):
                # project rows [sc*512 + j*128, +128)
                r0 = sc * 512 + j * 128
                oT_cur = oT_sb[sc % 2]
                for nch in range(2):
                    pp = ps.tile([128, 512], F32, tag="mm", bufs=4)
                    for hh in range(HPC):
                        nc.tensor.matmul(
                            pp[:],
                            oT_cur[:, hh * 512 + j * 128 : hh * 512 + (j + 1) * 128],
                            wpa[:, hh * C + nch * 512 : hh * C + (nch + 1) * 512],
                            start=(hh == 0), stop=(hh == HPC - 1),
                        )
                    pst = stage.tile([128, 512], F16, tag="pst", bufs=4)
                    nc.vector.tensor_copy(pst[:], pp[:])
                    nc.sync.dma_start(
                        partial_d[r0 : r0 + 128, nch * 512 : (nch + 1) * 512],
                        pst[:],
                    )

            def emit_rs(gi):
                r0, r1 = RS_GROUPS[gi]
                nc.gpsimd.collective_compute(
                    "ReduceScatter",
                    mybir.AluOpType.add,
                    replica_groups=[[0, 1, 2, 3], [4, 5, 6, 7]],
                    ins=[partial_d[r0:r1, :]],
                    outs=[rsout_d[gi][:]],
                )
                og = sum((b1 - b0) // 4 for (b0, b1) in RS_GROUPS[:gi])
                ln4 = (r1 - r0) // 4
                nc.sync.dma_start(out_part[og : og + ln4, :], rsout_d[gi][:])

            # ---------------- schedule ----------------
            for tt in range(8):
                emit_v(tt)
            for tch in (0, 1):
                for i in range(4):
                    emit_qk(i, tch)

            # att sc0, then sc1 with proj(sc0) interleaved between heads
            for h in range(HPC):
                emit_att_head(0, h)
            for h in range(HPC):
                emit_att_head(1, h)
                emit_proj_piece(0, h)

            # fin1 immediately (stall filled by two V units), then RS group 0
            emit_v(8)
            emit_v(9)
            for j in range(4):
                emit_proj_piece(1, j)
            emit_rs(0)

            # rest of phase B needed before att2
            for tt in range(10, 16):
                emit_v(tt)
            for i in range(4):
                emit_qk(i, 2)

            for h in range(HPC):
                emit_att_head(2, h)
            # fin2 (stall filled by two tch3 QK units), then RS group 1
            emit_qk(0, 3)
            emit_qk(1, 3)
            for j in range(4):
                emit_proj_piece(2, j)
            emit_rs(1)

            emit_qk(2, 3)
            emit_qk(3, 3)
            for h in range(HPC):
                emit_att_head(3, h)
            for j in range(4):
                emit_proj_piece(3, j)
            emit_rs(2)

            if DEBUG:
                nc.sync.dma_start(dbg["qkT"][:], qkT[:])
                nc.sync.dma_start(dbg["v_aug"][:], v_aug[:])
                nc.sync.dma_start(dbg["oT"][:, 0 : HPC * 512], oT_sb[0][:])
                nc.sync.dma_start(dbg["oT"][:, HPC * 512 :], oT_sb[1][:])
                nc.sync.dma_start(dbg["partial"][:], partial_d[:])

    nc.finalize()
    return nc


_NC = None


def _get_nc():
    global _NC
    if _NC is None:
        _NC = _build()
    return _NC


def _perm_qkv(w):
    # (..., h*192 + t*64 + c) -> (..., t*256 + h*64 + c)
    s = w.shape[:-1]
    return np.ascontiguousarray(
        w.reshape(*s, HPC, 3, HD).swapaxes(-3, -2).reshape(*s, CG)
    )


def _make_in_maps(x, Wqkv, bqkv, Wproj, bproj):
    x = np.asarray(x, dtype=np.float32)
    Wqkv = np.asarray(Wqkv, dtype=np.float32)
    bqkv = np.asarray(bqkv, dtype=np.float32)
    Wproj = np.asarray(Wproj, dtype=np.float32)
    bproj = np.asarray(bproj, dtype=np.float32)

    in_maps = []
    for c in range(N_CORES):
        b, g = divmod(c, 4)
        # x16: [p, w, kc, t] window-major transposed layout
        xT = x[b].T  # (C, T)
        x16 = np.ascontiguousarray(
            xT.reshape(KC, 128, 4, 512).transpose(1, 2, 0, 3).reshape(128, -1)
        ).astype(np.float16)
        wp_ = _perm_qkv(Wqkv[:, g * CG : (g + 1) * CG])
        w16 = np.ascontiguousarray(
            wp_.reshape(KC, 128, CG).transpose(1, 0, 2).reshape(128, -1)
        ).astype(np.float16)
        bq = _perm_qkv(bqkv[g * CG : (g + 1) * CG])
        qkb = np.ascontiguousarray(bq[:512].reshape(4, 128).T).astype(np.float32)
        vb = bq[512:768].reshape(1, 256).astype(np.float16)
        wpa = np.zeros((65, HPC * C), np.float32)
        for hh in range(HPC):
            wpa[0:64, hh * C : (hh + 1) * C] = Wproj[
                g * 256 + hh * 64 : g * 256 + (hh + 1) * 64, :
            ]
        if g == 0:
            wpa[64, 0:C] = bproj
        in_maps.append(
            {
                "x16": x16,
                "w16": w16,
                "qkb": qkb,
                "vb": vb,
                "wpa": wpa.astype(np.float16),
            }
        )
    return in_maps


def _run(in_maps, trace=False):
    nc = _get_nc()
    return run_bass_kernel_spmd(nc, in_maps, list(range(N_CORES)), trace=trace)


def kernel(x, Wqkv, bqkv, Wproj, bproj):
    in_maps = _make_in_maps(x, Wqkv, bqkv, Wproj, bproj)
    res = _run(in_maps)
    out = np.empty((B, T, C), np.float32)
    for c in range(N_CORES):
        b, g = divmod(c, 4)
        op = res.results[c]["out_part"].astype(np.float32)
        og = 0
        for r0, r1 in RS_GROUPS:
            ln4 = (r1 - r0) // 4
            out[b, r0 + g * ln4 : r0 + (g + 1) * ln4, :] = op[og : og + ln4]
            og += ln4
    return out
